# revision 19
# baseline (speedup 1.0000x reference)
"""MoE transformer MLP (top-2 of 8 experts) + log_softmax head, on 8 trn2 cores.

Sharding: expert parallelism with host-side token dispatch/combine (full
inputs arrive on host, so the gather happens during the host->device shard
upload -- no device collective needed). Core c computes, for its assigned
expert(s), phat[slot] = gelu(x_tok @ w1[e] + b1[e]) . w2sum[e] for a fixed
set of NB=17 blocks of 128 token-slots: 16 "main" blocks for its own expert
plus 1 overflow block that may serve a different (overloaded) expert via a
second resident weight matrix. The second GEMM of the MLP is algebraically
folded: the model output is log_softmax_S(sum_d y), and
sum_d (h @ w2[e] + b2[e]) = h . w2sum[e] + b2sum[e], so only w2sum[e] =
w2[e].sum(-1) is needed (computed on host) and the [T, D] expert outputs are
never materialized. The gate (0.03% of model FLOPs), top-2 routing, combine
y[t] = sum_e w_tok[t,e]*(phat+b2sum[e]) and the final log_softmax run on
host in float64.
"""

import os
import sys

for _p in ("/opt/trn_rl_repo",):
    if _p not in sys.path:
        sys.path.insert(0, _p)

import numpy as np

B, S, D, H, E, TOPK = 8, 1024, 512, 2048, 8, 2
T = B * S
KC = D // 128        # 4 contraction chunks
NH = H // 512        # 4 psum-bank-wide slices of H
NMAIN = 16           # main blocks (core's own expert), 2048 slots
NB = NMAIN + 1       # +1 overflow block with its own weight slot
CAP = NB * 128       # 2176 slots per core

_CACHE = {}
_LAST_RUN = None     # test.py reads this for the trace/exec time


def _build(has_b1: bool):
    import concourse.bass as bass  # noqa: F401
    import concourse.tile as tile
    import concourse.mybir as mybir
    from concourse import bacc

    dt = mybir.dt
    f32 = dt.float32
    f16 = dt.float16
    ALU = mybir.AluOpType
    ACT = mybir.ActivationFunctionType

    nc = bacc.Bacc(None, target_bir_lowering=False)

    with tile.TileContext(nc) as tc:
        with tc.tile_pool(name="dram", bufs=1, space="DRAM") as dram:
            # host arrays are laid out partition-major, exactly matching SBUF
            xT_d = dram.tile([128, NB, KC, 128], f16, kind="ExternalInput", name="xT", uniquify=False)
            w_d = dram.tile([128, 2, KC, H], f16, kind="ExternalInput", name="w1g", uniquify=False)
            w2s_d = dram.tile([128, 2, H], f16, kind="ExternalInput", name="w2s", uniquify=False)
            if has_b1:
                b1_d = dram.tile([2, H], f32, kind="ExternalInput", name="b1g", uniquify=False)
            out_d = dram.tile([128, NB + NH - 1], f32, kind="ExternalOutput", name="out", uniquify=False)

            with tc.tile_pool(name="singles", bufs=1) as singles:
                xT_sb = singles.tile([128, NB, KC, 128], f16)
                w_sb = singles.tile([128, 2, KC, H], f16)
                w2s_sb = singles.tile([128, 2, H], f16)
                phat = singles.tile([128, NB + NH - 1], f32)
                scratch = singles.tile([128, 512], f16)
                nc.vector.memset(scratch, 0.25)
                if has_b1:
                    ones_row = singles.tile([1, 128], f32)
                    nc.vector.memset(ones_row, 1.0)
                    b1_sb = singles.tile([1, 2, H], f32)
                    nc.scalar.dma_start(out=b1_sb, in_=b1_d[None])

                # single sync HWDGE queue, emitted in exact consumption
                # order (queue transfers run serially at ~400GB/s; one
                # doorbell costs ~700ns, so medium granularity).
                nc.sync.dma_start(out=xT_sb[:, 0], in_=xT_d[:, 0])
                nc.sync.dma_start(out=w_sb[:, 0, 0, :1024], in_=w_d[:, 0, 0, :1024])
                nc.sync.dma_start(out=w_sb[:, 0, 0, 1024:], in_=w_d[:, 0, 0, 1024:])
                nc.sync.dma_start(out=w_sb[:, 0, 1, :], in_=w_d[:, 0, 1, :])
                nc.sync.dma_start(out=xT_sb[:, 1:3], in_=xT_d[:, 1:3])
                nc.sync.dma_start(out=w_sb[:, 0, 2, :], in_=w_d[:, 0, 2, :])
                nc.sync.dma_start(out=w_sb[:, 0, 3, :], in_=w_d[:, 0, 3, :])
                nc.sync.dma_start(out=w2s_sb[:, 0, :], in_=w2s_d[:, 0, :])
                nc.sync.dma_start(out=xT_sb[:, 3:7], in_=xT_d[:, 3:7])
                nc.sync.dma_start(out=xT_sb[:, 7:12], in_=xT_d[:, 7:12])
                nc.sync.dma_start(out=w_sb[:, 1], in_=w_d[:, 1])
                nc.sync.dma_start(out=w2s_sb[:, 1, :], in_=w2s_d[:, 1, :])
                nc.sync.dma_start(out=xT_sb[:, 12:NB], in_=xT_d[:, 12:NB])

                # warm the PE clock (HAM) during the DMA wait: ~3us of
                # throwaway matmuls that end right as the first data lands
                with tc.tile_pool(name="warm", bufs=1, space="PSUM") as warm:
                    wp = warm.tile([128, 512], f32)
                    for _ in range(7):
                        nc.tensor.matmul(wp, scratch[:, :128], scratch,
                                         start=True, stop=True,
                                         skip_group_check=True)

                with tc.tile_pool(name="gp", bufs=3) as gp, \
                     tc.tile_pool(name="psm", bufs=2, space="PSUM") as psm:
                    for b in range(NB):
                        s = 0 if b < NMAIN else 1
                        hp = psm.tile([128, H], f32, tag="hp")
                        if b < NB - 1:
                            for k in range(KC):
                                lhsT = xT_sb[:, b, k, :]
                                for n in range(NH):
                                    nc.tensor.matmul(
                                        hp[:, n * 512:(n + 1) * 512], lhsT,
                                        w_sb[:, s, k, n * 512:(n + 1) * 512],
                                        start=(k == 0),
                                        stop=(k == KC - 1 and not has_b1))
                            if has_b1:
                                for n in range(NH):
                                    nc.tensor.matmul(
                                        hp[:, n * 512:(n + 1) * 512], ones_row,
                                        b1_sb[:, s, n * 512:(n + 1) * 512],
                                        start=False, stop=True)
                            g = gp.tile([128, H], f16, tag="g")
                            nc.scalar.activation(out=g, in_=hp, func=ACT.Gelu)
                            # phat[slot, b] = sum_h g * w2sum (fused mult+accum)
                            nc.vector.scalar_tensor_tensor(
                                out=g, in0=g, scalar=1.0, in1=w2s_sb[:, s, :],
                                op0=ALU.mult, op1=ALU.mult,
                                accum_out=phat[:, b:b + 1])
                        else:
                            # last block: n-outer/k-inner + per-column
                            # gelu/dot, so the tail chain overlaps the
                            # block's own matmuls; host sums the 4 partials.
                            for n in range(NH):
                                hcol = hp[:, n * 512:(n + 1) * 512]
                                for k in range(KC):
                                    nc.tensor.matmul(
                                        hcol, xT_sb[:, b, k, :],
                                        w_sb[:, s, k, n * 512:(n + 1) * 512],
                                        start=(k == 0),
                                        stop=(k == KC - 1 and not has_b1))
                                if has_b1:
                                    nc.tensor.matmul(
                                        hcol, ones_row,
                                        b1_sb[:, s, n * 512:(n + 1) * 512],
                                        start=False, stop=True)
                                gc = gp.tile([128, 512], f16, tag="gc")
                                nc.scalar.activation(out=gc, in_=hcol,
                                                     func=ACT.Gelu)
                                nc.vector.scalar_tensor_tensor(
                                    out=gc, in0=gc, scalar=1.0,
                                    in1=w2s_sb[:, s, n * 512:(n + 1) * 512],
                                    op0=ALU.mult, op1=ALU.mult,
                                    accum_out=phat[:, b + n:b + n + 1])

                nc.sync.dma_start(out=out_d[:, :NMAIN], in_=phat[:, :NMAIN])
                nc.sync.dma_start(out=out_d[:, NMAIN:], in_=phat[:, NMAIN:])

    nc.compile()
    return nc


def get_nc(has_b1: bool):
    key = bool(has_b1)
    if key not in _CACHE:
        _CACHE[key] = _build(key)
    return _CACHE[key]


def _gelu_exact(z):
    try:
        from scipy.special import erf
    except Exception:
        import math
        erf = np.frompyfunc(math.erf, 1, 1)
    return 0.5 * z * (1.0 + np.asarray(erf(z / np.sqrt(2.0)), np.float64))


def route(x, gate_w, gate_b):
    """Host gate: returns top-2 expert ids [T,2] and combine weights [T,2]."""
    xt = np.asarray(x, np.float32).reshape(T, D)
    logits = xt @ np.asarray(gate_w, np.float32) + np.asarray(gate_b, np.float32)
    top = np.argsort(-logits, axis=1, kind="stable")[:, :TOPK]
    v = np.take_along_axis(logits, top, axis=1).astype(np.float64)
    e = np.exp(v - v.max(axis=1, keepdims=True))
    sc = e / e.sum(axis=1, keepdims=True)
    return top.astype(np.int32), sc


def prep(x, gate_w, gate_b, w1, b1, w2, b2):
    """Build per-core in_maps + the combine context."""
    f = np.float32
    x = np.asarray(x, f)
    xt = x.reshape(T, D)
    top, sc = route(x, gate_w, gate_b)

    # slot lists per expert: (token id, combine weight)
    tok_of = [np.where((top == e).any(axis=1))[0] for e in range(E)]
    wt_of = []
    for e in range(E):
        tk = tok_of[e]
        is1 = top[tk, 0] == e
        wt_of.append(np.where(is1, sc[tk, 0], sc[tk, 1]))

    # core e: first min(count, 2048) tokens of expert e; overflow in chunks
    # of <=128 goes to other cores' extra block (one per core).
    core_main = []
    chunks = []  # (expert, toks, wts)
    host_left = []  # (expert, toks, wts) computed on host if >8 chunks
    for e in range(E):
        tk, wt = tok_of[e], wt_of[e]
        core_main.append((tk[:NMAIN * 128], wt[:NMAIN * 128]))
        rest_t, rest_w = tk[NMAIN * 128:], wt[NMAIN * 128:]
        for i in range(0, len(rest_t), 128):
            chunks.append((e, rest_t[i:i + 128], rest_w[i:i + 128]))
    if len(chunks) > E:
        host_left = chunks[E:]
        chunks = chunks[:E]

    w1f = np.asarray(w1, f)
    w2sum = np.asarray(w2, f).sum(axis=2, dtype=np.float64).astype(f)  # [E, H]
    b1f = np.asarray(b1, f)
    has_b1 = bool(np.any(b1f))

    in_maps = []
    slot_tok = np.full((B, CAP), -1, np.int64)
    slot_wt = np.zeros((B, CAP), np.float64)
    for c in range(B):
        mt, mw = core_main[c]
        slot_tok[c, :len(mt)] = mt
        slot_wt[c, :len(mt)] = mw
        if c < len(chunks):
            xe, xt_ids, xw = chunks[c]
        else:
            xe = c
            xt_ids = np.empty(0, np.int64)
            xw = np.empty(0, np.float64)
        slot_tok[c, NMAIN * 128:NMAIN * 128 + len(xt_ids)] = xt_ids
        slot_wt[c, NMAIN * 128:NMAIN * 128 + len(xt_ids)] = xw

        gather = np.where(slot_tok[c] >= 0, slot_tok[c], 0)
        xg = xt[gather]                                   # [CAP, D]
        # partition-major [p, b, k, q], matching the SBUF tile layout
        xT = np.ascontiguousarray(
            xg.reshape(NB, 128, KC, 128).transpose(3, 0, 2, 1)).astype(np.float16)
        sel = [c, xe]
        wg = np.ascontiguousarray(
            w1f[sel].reshape(2, KC, 128, H).transpose(2, 0, 1, 3)).astype(np.float16)
        w2sg = np.ascontiguousarray(
            np.broadcast_to(w2sum[sel].astype(np.float16)[None], (128, 2, H)))
        m = {"xT": xT, "w1g": wg, "w2s": w2sg}
        if has_b1:
            m["b1g"] = np.ascontiguousarray(b1f[sel])
        in_maps.append(m)

    ctx = {
        "slot_tok": slot_tok, "slot_wt": slot_wt,
        "top": top, "sc": sc, "host_left": host_left,
        "b2sum": np.asarray(b2, f).sum(axis=1, dtype=np.float64),
        "xt": xt, "w1f": w1f, "b1f": b1f, "w2sum": w2sum,
        "has_b1": has_b1,
    }
    return in_maps, ctx


def combine(phats, ctx):
    """phats: list of per-core [CAP] f32. Returns [B, S] f32 log_softmax."""
    y = np.zeros(T, np.float64)
    for c in range(B):
        valid = ctx["slot_tok"][c] >= 0
        np.add.at(y, ctx["slot_tok"][c][valid],
                  ctx["slot_wt"][c][valid] * np.asarray(phats[c], np.float64)[valid])
    for e, tk, wt in ctx["host_left"]:  # exact host fallback (rare/never)
        z = ctx["xt"][tk].astype(np.float64) @ ctx["w1f"][e].astype(np.float64)
        if ctx["has_b1"]:
            z = z + ctx["b1f"][e]
        ph = _gelu_exact(z) @ ctx["w2sum"][e].astype(np.float64)
        np.add.at(y, tk, wt * ph)
    top, sc, b2s = ctx["top"], ctx["sc"], ctx["b2sum"]
    y += (sc[:, 0] * b2s[top[:, 0]]) + (sc[:, 1] * b2s[top[:, 1]])
    y = y.reshape(B, S)
    m = y.max(axis=1, keepdims=True)
    out = y - (m + np.log(np.exp(y - m).sum(axis=1, keepdims=True)))
    return out.astype(np.float32)


def kernel(x, gate_w, gate_b, w1, b1, w2, b2):
    global _LAST_RUN
    from concourse.bass_utils import run_bass_kernel_spmd

    in_maps, ctx = prep(x, gate_w, gate_b, w1, b1, w2, b2)
    nc = get_nc(ctx["has_b1"])
    trace = os.environ.get("KTRACE", "0") == "1"
    res = run_bass_kernel_spmd(nc, in_maps, core_ids=list(range(B)), trace=trace)
    _LAST_RUN = res
    # device out is [128, NB+3]: cols 0..NB-2 are blocks, cols NB-1..NB+2
    # are the last block's 4 H-column partials; slot index is b*128 + p
    phats = []
    for c in range(B):
        arr = np.asarray(res.results[c]["out"], np.float32)
        ph = np.empty((128, NB), np.float32)
        ph[:, :NB - 1] = arr[:, :NB - 1]
        ph[:, NB - 1] = arr[:, NB - 1:].sum(axis=1)
        phats.append(ph.T.reshape(CAP))
    return combine(phats, ctx)


# revision 23
# speedup vs baseline: 1.0491x; 1.0491x over previous
"""MoE transformer MLP (top-2 of 8 experts) + log_softmax head, on 8 trn2 cores.

Sharding: expert parallelism with host-side token dispatch/combine (full
inputs arrive on host, so the gather happens during the host->device shard
upload -- no device collective needed). Core c computes, for its assigned
expert(s), phat[slot] = gelu(x_tok @ w1[e] + b1[e]) . w2sum[e] for a fixed
set of NB=17 blocks of 128 token-slots: 16 "main" blocks for its own expert
plus 1 overflow block that may serve a different (overloaded) expert via a
second resident weight matrix. The second GEMM of the MLP is algebraically
folded: the model output is log_softmax_S(sum_d y), and
sum_d (h @ w2[e] + b2[e]) = h . w2sum[e] + b2sum[e], so only w2sum[e] =
w2[e].sum(-1) is needed (computed on host) and the [T, D] expert outputs are
never materialized. The gate (0.03% of model FLOPs), top-2 routing, combine
y[t] = sum_e w_tok[t,e]*(phat+b2sum[e]) and the final log_softmax run on
host in float64.
"""

import os
import sys

for _p in ("/opt/trn_rl_repo",):
    if _p not in sys.path:
        sys.path.insert(0, _p)

import numpy as np

B, S, D, H, E, TOPK = 8, 1024, 512, 2048, 8, 2
T = B * S
KC = D // 128        # 4 contraction chunks
NH = H // 512        # 4 psum-bank-wide slices of H
NMAIN = 16           # main blocks (core's own expert), 2048 slots
NB = NMAIN + 1       # +1 overflow block with its own weight slot
CAP = NB * 128       # 2176 slots per core

_CACHE = {}
_LAST_RUN = None     # test.py reads this for the trace/exec time


def _build(has_b1: bool):
    import concourse.bass as bass  # noqa: F401
    import concourse.tile as tile
    import concourse.mybir as mybir
    from concourse import bacc

    dt = mybir.dt
    f32 = dt.float32
    f16 = dt.float16
    ALU = mybir.AluOpType
    ACT = mybir.ActivationFunctionType

    nc = bacc.Bacc(None, target_bir_lowering=False)

    with tile.TileContext(nc) as tc:
        with tc.tile_pool(name="dram", bufs=1, space="DRAM") as dram:
            # host arrays are laid out partition-major, exactly matching SBUF
            xT_d = dram.tile([128, NB, KC, 128], f16, kind="ExternalInput", name="xT", uniquify=False)
            w_d = dram.tile([128, 2, KC, H], f16, kind="ExternalInput", name="w1g", uniquify=False)
            w2s_d = dram.tile([128, 2, H], f16, kind="ExternalInput", name="w2s", uniquify=False)
            if has_b1:
                b1_d = dram.tile([2, H], f32, kind="ExternalInput", name="b1g", uniquify=False)
            out_d = dram.tile([128, NB + NH - 1], f32, kind="ExternalOutput", name="out", uniquify=False)

            with tc.tile_pool(name="singles", bufs=1) as singles:
                xT_sb = singles.tile([128, NB, KC, 128], f16)
                w_sb = singles.tile([128, 2, KC, H], f16)
                w2s_sb = singles.tile([128, 2, H], f16)
                phat = singles.tile([128, NB + NH - 1], f32)

                if has_b1:
                    ones_row = singles.tile([1, 128], f32)
                    nc.vector.memset(ones_row, 1.0)
                    b1_sb = singles.tile([1, 2, H], f32)
                    nc.scalar.dma_start(out=b1_sb, in_=b1_d[None])

                # single sync HWDGE queue, emitted in exact consumption
                # order (queue transfers run serially at ~400GB/s; one
                # doorbell costs ~700ns, so medium granularity).
                nc.sync.dma_start(out=xT_sb[:, 0], in_=xT_d[:, 0])
                nc.sync.dma_start(out=w_sb[:, 0, 0, :], in_=w_d[:, 0, 0, :])
                nc.sync.dma_start(out=w_sb[:, 0, 1, :], in_=w_d[:, 0, 1, :])
                nc.sync.dma_start(out=xT_sb[:, 1:3], in_=xT_d[:, 1:3])
                nc.sync.dma_start(out=w_sb[:, 0, 2, :], in_=w_d[:, 0, 2, :])
                nc.sync.dma_start(out=w_sb[:, 0, 3, :], in_=w_d[:, 0, 3, :])
                nc.sync.dma_start(out=w2s_sb[:, 0, :], in_=w2s_d[:, 0, :])
                nc.sync.dma_start(out=xT_sb[:, 3:7], in_=xT_d[:, 3:7])
                nc.sync.dma_start(out=xT_sb[:, 7:12], in_=xT_d[:, 7:12])
                nc.sync.dma_start(out=w_sb[:, 1], in_=w_d[:, 1])
                nc.sync.dma_start(out=w2s_sb[:, 1, :], in_=w2s_d[:, 1, :])
                nc.sync.dma_start(out=xT_sb[:, 12:NB], in_=xT_d[:, 12:NB])

                with tc.tile_pool(name="gp", bufs=3) as gp, \
                     tc.tile_pool(name="psm", bufs=2, space="PSUM") as psm:
                    for b in range(NB):
                        s = 0 if b < NMAIN else 1
                        hp = psm.tile([128, H], f32, tag="hp")
                        if b < NB - 1:
                            for k in range(KC):
                                lhsT = xT_sb[:, b, k, :]
                                for n in range(NH):
                                    nc.tensor.matmul(
                                        hp[:, n * 512:(n + 1) * 512], lhsT,
                                        w_sb[:, s, k, n * 512:(n + 1) * 512],
                                        start=(k == 0),
                                        stop=(k == KC - 1 and not has_b1))
                            if has_b1:
                                for n in range(NH):
                                    nc.tensor.matmul(
                                        hp[:, n * 512:(n + 1) * 512], ones_row,
                                        b1_sb[:, s, n * 512:(n + 1) * 512],
                                        start=False, stop=True)
                            g = gp.tile([128, H], f16, tag="g")
                            nc.scalar.activation(out=g, in_=hp, func=ACT.Gelu)
                            # phat[slot, b] = sum_h g * w2sum (fused mult+accum)
                            nc.vector.scalar_tensor_tensor(
                                out=g, in0=g, scalar=1.0, in1=w2s_sb[:, s, :],
                                op0=ALU.mult, op1=ALU.mult,
                                accum_out=phat[:, b:b + 1])
                        else:
                            # last block: per-column gelu/dot so the tail
                            # chain starts as each column's accumulation
                            # finishes; host sums the 4 partials.
                            for k in range(KC):
                                lhsT = xT_sb[:, b, k, :]
                                for n in range(NH):
                                    nc.tensor.matmul(
                                        hp[:, n * 512:(n + 1) * 512], lhsT,
                                        w_sb[:, s, k, n * 512:(n + 1) * 512],
                                        start=(k == 0),
                                        stop=(k == KC - 1 and not has_b1))
                            if has_b1:
                                for n in range(NH):
                                    nc.tensor.matmul(
                                        hp[:, n * 512:(n + 1) * 512], ones_row,
                                        b1_sb[:, s, n * 512:(n + 1) * 512],
                                        start=False, stop=True)
                            for n in range(NH):
                                gc = gp.tile([128, 512], f16, tag="gc")
                                nc.scalar.activation(
                                    out=gc, in_=hp[:, n * 512:(n + 1) * 512],
                                    func=ACT.Gelu)
                                nc.vector.scalar_tensor_tensor(
                                    out=gc, in0=gc, scalar=1.0,
                                    in1=w2s_sb[:, s, n * 512:(n + 1) * 512],
                                    op0=ALU.mult, op1=ALU.mult,
                                    accum_out=phat[:, b + n:b + n + 1])

                nc.sync.dma_start(out=out_d[:, :NMAIN], in_=phat[:, :NMAIN])
                nc.sync.dma_start(out=out_d[:, NMAIN:], in_=phat[:, NMAIN:])

    nc.compile()
    return nc


def get_nc(has_b1: bool):
    key = bool(has_b1)
    if key not in _CACHE:
        _CACHE[key] = _build(key)
    return _CACHE[key]


def _gelu_exact(z):
    try:
        from scipy.special import erf
    except Exception:
        import math
        erf = np.frompyfunc(math.erf, 1, 1)
    return 0.5 * z * (1.0 + np.asarray(erf(z / np.sqrt(2.0)), np.float64))


def route(x, gate_w, gate_b):
    """Host gate: returns top-2 expert ids [T,2] and combine weights [T,2]."""
    xt = np.asarray(x, np.float32).reshape(T, D)
    logits = xt @ np.asarray(gate_w, np.float32) + np.asarray(gate_b, np.float32)
    top = np.argsort(-logits, axis=1, kind="stable")[:, :TOPK]
    v = np.take_along_axis(logits, top, axis=1).astype(np.float64)
    e = np.exp(v - v.max(axis=1, keepdims=True))
    sc = e / e.sum(axis=1, keepdims=True)
    return top.astype(np.int32), sc


def prep(x, gate_w, gate_b, w1, b1, w2, b2):
    """Build per-core in_maps + the combine context."""
    f = np.float32
    x = np.asarray(x, f)
    xt = x.reshape(T, D)
    top, sc = route(x, gate_w, gate_b)

    # slot lists per expert: (token id, combine weight)
    tok_of = [np.where((top == e).any(axis=1))[0] for e in range(E)]
    wt_of = []
    for e in range(E):
        tk = tok_of[e]
        is1 = top[tk, 0] == e
        wt_of.append(np.where(is1, sc[tk, 0], sc[tk, 1]))

    # core e: first min(count, 2048) tokens of expert e; overflow in chunks
    # of <=128 goes to other cores' extra block (one per core).
    core_main = []
    chunks = []  # (expert, toks, wts)
    host_left = []  # (expert, toks, wts) computed on host if >8 chunks
    for e in range(E):
        tk, wt = tok_of[e], wt_of[e]
        core_main.append((tk[:NMAIN * 128], wt[:NMAIN * 128]))
        rest_t, rest_w = tk[NMAIN * 128:], wt[NMAIN * 128:]
        for i in range(0, len(rest_t), 128):
            chunks.append((e, rest_t[i:i + 128], rest_w[i:i + 128]))
    if len(chunks) > E:
        host_left = chunks[E:]
        chunks = chunks[:E]

    w1f = np.asarray(w1, f)
    w2sum = np.asarray(w2, f).sum(axis=2, dtype=np.float64).astype(f)  # [E, H]
    b1f = np.asarray(b1, f)
    has_b1 = bool(np.any(b1f))

    in_maps = []
    slot_tok = np.full((B, CAP), -1, np.int64)
    slot_wt = np.zeros((B, CAP), np.float64)
    for c in range(B):
        mt, mw = core_main[c]
        slot_tok[c, :len(mt)] = mt
        slot_wt[c, :len(mt)] = mw
        if c < len(chunks):
            xe, xt_ids, xw = chunks[c]
        else:
            xe = c
            xt_ids = np.empty(0, np.int64)
            xw = np.empty(0, np.float64)
        slot_tok[c, NMAIN * 128:NMAIN * 128 + len(xt_ids)] = xt_ids
        slot_wt[c, NMAIN * 128:NMAIN * 128 + len(xt_ids)] = xw

        gather = np.where(slot_tok[c] >= 0, slot_tok[c], 0)
        xg = xt[gather]                                   # [CAP, D]
        # partition-major [p, b, k, q], matching the SBUF tile layout
        xT = np.ascontiguousarray(
            xg.reshape(NB, 128, KC, 128).transpose(3, 0, 2, 1)).astype(np.float16)
        sel = [c, xe]
        wg = np.ascontiguousarray(
            w1f[sel].reshape(2, KC, 128, H).transpose(2, 0, 1, 3)).astype(np.float16)
        w2sg = np.ascontiguousarray(
            np.broadcast_to(w2sum[sel].astype(np.float16)[None], (128, 2, H)))
        m = {"xT": xT, "w1g": wg, "w2s": w2sg}
        if has_b1:
            m["b1g"] = np.ascontiguousarray(b1f[sel])
        in_maps.append(m)

    ctx = {
        "slot_tok": slot_tok, "slot_wt": slot_wt,
        "top": top, "sc": sc, "host_left": host_left,
        "b2sum": np.asarray(b2, f).sum(axis=1, dtype=np.float64),
        "xt": xt, "w1f": w1f, "b1f": b1f, "w2sum": w2sum,
        "has_b1": has_b1,
    }
    return in_maps, ctx


def combine(phats, ctx):
    """phats: list of per-core [CAP] f32. Returns [B, S] f32 log_softmax."""
    y = np.zeros(T, np.float64)
    for c in range(B):
        valid = ctx["slot_tok"][c] >= 0
        np.add.at(y, ctx["slot_tok"][c][valid],
                  ctx["slot_wt"][c][valid] * np.asarray(phats[c], np.float64)[valid])
    for e, tk, wt in ctx["host_left"]:  # exact host fallback (rare/never)
        z = ctx["xt"][tk].astype(np.float64) @ ctx["w1f"][e].astype(np.float64)
        if ctx["has_b1"]:
            z = z + ctx["b1f"][e]
        ph = _gelu_exact(z) @ ctx["w2sum"][e].astype(np.float64)
        np.add.at(y, tk, wt * ph)
    top, sc, b2s = ctx["top"], ctx["sc"], ctx["b2sum"]
    y += (sc[:, 0] * b2s[top[:, 0]]) + (sc[:, 1] * b2s[top[:, 1]])
    y = y.reshape(B, S)
    m = y.max(axis=1, keepdims=True)
    out = y - (m + np.log(np.exp(y - m).sum(axis=1, keepdims=True)))
    return out.astype(np.float32)


def kernel(x, gate_w, gate_b, w1, b1, w2, b2):
    global _LAST_RUN
    from concourse.bass_utils import run_bass_kernel_spmd

    in_maps, ctx = prep(x, gate_w, gate_b, w1, b1, w2, b2)
    nc = get_nc(ctx["has_b1"])
    trace = os.environ.get("KTRACE", "0") == "1"
    res = run_bass_kernel_spmd(nc, in_maps, core_ids=list(range(B)), trace=trace)
    _LAST_RUN = res
    # device out is [128, NB+3]: cols 0..NB-2 are blocks, cols NB-1..NB+2
    # are the last block's 4 H-column partials; slot index is b*128 + p
    phats = []
    for c in range(B):
        arr = np.asarray(res.results[c]["out"], np.float32)
        ph = np.empty((128, NB), np.float32)
        ph[:, :NB - 1] = arr[:, :NB - 1]
        ph[:, NB - 1] = arr[:, NB - 1:].sum(axis=1)
        phats.append(ph.T.reshape(CAP))
    return combine(phats, ctx)
